# revision 56
# baseline (speedup 1.0000x reference)
# Differential GQA attention layer (B=2, S=1024, E=2048, H=16, KVH=4, D=128)
# distributed over 8 TRN2 NeuronCores: shard = (batch b, kv-group g) so each
# core owns 1 batch x 4 query heads (1 kv head). All attention is core-local;
# the Wo row-sharded output projection partials are summed on the host.
#
# Software-pipelined schedule (~224us vs 275us for the phase-serial version):
# per-head attention (softmax chain on scalar/vector) overlaps the NEXT
# head's q-projection matmuls on the PE, so the tensor engine never drains
# between projection and attention. Energy psums are wide [128,1024] tiles so
# each exp half is ONE activation with accum_out row-sums; causal diag masks
# are PE-accumulated (identity x negU matmul); per-row softmax scalars are
# batched per half-head (tiny DVE reciprocals - large ones cost ~6.4ns/elem);
# scalar engine runs exp exclusively during stages (act-table switches cost
# 1.3us); the RMS rsqrt is ONE compact recip+sqrt on a [128,16] strip per
# q-half, broadcast across partitions with selector matmuls; v is produced in
# [pos,d] layout by a wide DMA transpose; x/w SBUF and the projection psum
# banks are recycled after stage 2 so out-proj for q<512 overlaps stage 3.
#
# Self-contained: hardcodes shapes/sharding; builds+compiles a Bass/Tile
# kernel on first call and runs it via run_bass_kernel_spmd on cores 0-7.
import numpy as np

B, S, E, H, KVH = 2, 1024, 2048, 16, 4
D = 128
NEG = -1e30
LAM_INIT = 0.2  # 0.8 - 0.6*exp(-0.3*layer_idx), layer_idx=0
NCORES = 8
HPC = H // KVH  # heads per core = 4
NQT = S // 128  # 8 q/k position blocks
NBLK = 2 + 1 + 2 * HPC  # weight column blocks: k0,k1,v,q0..q7

_cache = {}


def _build(dbg=False):
    import concourse.mybir as mybir
    import concourse.tile as tile
    from concourse import bacc
    from concourse.masks import make_identity
    from contextlib import ExitStack

    F32 = mybir.dt.float32
    BF16 = mybir.dt.bfloat16
    ALU = mybir.AluOpType
    ACT = mybir.ActivationFunctionType

    nc = bacc.Bacc(None, target_bir_lowering=False)

    # pre-split by position half on the host so each [128,512] tile load
    # reads contiguous 1KB dram rows (full-row xT loads would be strided)
    xT = nc.declare_dram_parameter("xT", [2 * E, S // 2], BF16, isOutput=False)
    # all projection weights, e-interleaved per 128-col block:
    # Wil[p, blk*2048 + e*128 + c] = W[e*128+p, blk_cols+c]
    Wil = nc.declare_dram_parameter("Wil", [128, NBLK * 2048], BF16, isOutput=False)
    Wo = nc.declare_dram_parameter("Wo", [HPC * D, E], BF16, isOutput=False)
    cosd = nc.declare_dram_parameter("cosd", [2 * D, S], BF16, isOutput=False)
    sind = nc.declare_dram_parameter("sind", [2 * D, S], BF16, isOutput=False)
    lamn = nc.declare_dram_parameter("lamn", [D, HPC], F32, isOutput=False)
    maskn = nc.declare_dram_parameter("maskn", [D, D], F32, isOutput=False)
    out_ext = nc.declare_dram_parameter("out", [S, E], BF16, isOutput=True)

    ISCALE = 1.0 / float(np.sqrt(D))

    with tile.TileContext(nc) as tc:
        ctx = ExitStack()
        with ctx:
            cpool = ctx.enter_context(tc.tile_pool(name="const", bufs=1))
            qkpool = ctx.enter_context(tc.tile_pool(name="qk", bufs=1))
            epool = ctx.enter_context(tc.tile_pool(name="expp", bufs=3))
            smp = ctx.enter_context(tc.tile_pool(name="sm", bufs=2))
            dtpool = ctx.enter_context(tc.tile_pool(name="difft", bufs=2))
            pvps = ctx.enter_context(tc.tile_pool(name="pvps", bufs=1, space="PSUM"))
            ssqps = ctx.enter_context(tc.tile_pool(name="ssq", bufs=1, space="PSUM"))
            # psum: 2 (proj) + 2x2 (energy) + 1 (pv) + 1 (ssq) = 8 banks.
            # m1ps+xpool+wpool are LIFO-closed after stage 2 (projections done);
            # their 2 psum banks become the out-projection accumulators, which
            # lets out-proj p=0..3 run inside stage 3.
            engps = ctx.enter_context(tc.tile_pool(name="engps", bufs=2, space="PSUM"))
            pctx = ExitStack()
            m1ps = pctx.enter_context(tc.tile_pool(name="m1ps", bufs=2, space="PSUM"))
            xpool = pctx.enter_context(tc.tile_pool(name="xT", bufs=32))
            wpool = pctx.enter_context(tc.tile_pool(name="w", bufs=3))

            # ---------------- constants ----------------
            cos_b = [cpool.tile([128, S], BF16, tag=f"cosb{a}", name=f"cosb{a}") for a in range(2)]
            sin_b = [cpool.tile([128, S], BF16, tag=f"sinb{a}", name=f"sinb{a}") for a in range(2)]
            lam_t = cpool.tile([128, HPC], F32, tag="lam", name="lam")
            maskf = cpool.tile([128, 128], F32, tag="maskf", name="maskf")
            negU = cpool.tile([128, 128], BF16, tag="negU", name="negU")
            identb = cpool.tile([128, 128], BF16, tag="identb", name="identb")
            ones_t = cpool.tile([128, 128], BF16, tag="ones", name="ones")

            identf = cpool.tile([128, 128], F32, tag="identf", name="identf")
            sel = cpool.tile([32, 32 * 128], BF16, tag="sel", name="sel")

            def load_consts():
                for a in range(2):
                    nc.gpsimd.dma_start(out=cos_b[a][:], in_=cosd[a * 128:(a + 1) * 128, :])
                    nc.gpsimd.dma_start(out=sin_b[a][:], in_=sind[a * 128:(a + 1) * 128, :])
                nc.gpsimd.dma_start(out=lam_t[:], in_=lamn[:])
                nc.gpsimd.dma_start(out=maskf[:], in_=maskn[:])
                nc.vector.tensor_copy(negU[:], maskf[:])

            def build_consts():
                # emitted AFTER the input DMAs so they don't delay the fill
                make_identity(nc, identb[:])
                make_identity(nc, identf[:])
                nc.gpsimd.memset(ones_t[:], 1.0)
                # row-selector for the tail cfac broadcast:
                # sel[k, col*128+m] = 1 iff k == col  (col < 32)
                for col in range(32):
                    nc.vector.tensor_scalar(
                        sel[:, col * 128:(col + 1) * 128], ones_t[0:32, 0:128],
                        identf[0:32, col:col + 1], None, op0=ALU.mult)

            # ---------------- persistent activations ----------------
            # x^T in position halves so the first projection chunk only waits
            # on 2MB of DMA instead of 4MB
            xt = [[xpool.tile([128, 512], BF16, tag="xt", name=f"xt{e}_{p}")
                   for p in range(2)] for e in range(16)]
            # per-(q-position) RMS sums, col = h*8 + c*4 + qb, written by N=1
            # matmuls during the stages, consumed once in the tail
            ssq = ssqps.tile([128, 128], F32, tag="ssq", name="ssq")
            qT = [[qkpool.tile([128, S], BF16, tag=f"qT{h}{a}", name=f"qT{h}{a}")
                   for a in range(2)] for h in range(HPC)]
            kT = [qkpool.tile([128, S], BF16, tag=f"kT{a}", name=f"kT{a}") for a in range(2)]
            v4 = [qkpool.tile([128, 4, 128], BF16, tag=f"v4{j}", name=f"v4{j}")
                  for j in range(2)]
            wo_t = [qkpool.tile([128, E], BF16, tag=f"wo{h}", name=f"wo{h}")
                    for h in range(HPC)]
            attr = [[qkpool.tile([128, 512], BF16, tag=f"attr{h}{c}", name=f"attr{h}{c}")
                     for c in range(2)] for h in range(HPC)]

            # ---------------- DMA helpers ----------------
            XQ = [nc.sync, nc.scalar, nc.gpsimd]

            def load_x():
                # p-major so chunk p=0 of all e lands first
                for p in range(2):
                    for e in range(16):
                        XQ[e % 3].dma_start(
                            out=xt[e][p][:],
                            in_=xT[p * E + e * 128:p * E + (e + 1) * 128, :])

            def load_wb(bi, fine=False):
                wb = wpool.tile([128, 2048], BF16, tag="w", name=f"w{bi}")
                if fine:
                    for g4 in range(4):
                        nc.sync.dma_start(
                            out=wb[:, g4 * 512:(g4 + 1) * 512],
                            in_=Wil[:, bi * 2048 + g4 * 512:bi * 2048 + (g4 + 1) * 512])
                else:
                    nc.sync.dma_start(out=wb[:], in_=Wil[:, bi * 2048:(bi + 1) * 2048])
                return wb

            # ---------------- projection + rope ----------------
            def proj_chunk(wb, p):
                """16 accumulating matmuls for position chunk p -> psum [128,512]."""
                ps = m1ps.tile([128, 512], F32, tag="m1", name="m1")
                for e in range(16):
                    nc.tensor.matmul(ps[:], wb[:, e * 128:(e + 1) * 128],
                                     xt[e][p][:], start=(e == 0), stop=(e == 15))
                return ps

            def rope_chunk(dst, a, ps, p):
                """dst[:, p-chunk] = ps*cos + swap128(ps)*sin (emits on
                scalar/vector/gpsimd; reads proj psum)."""
                sl = slice(p * 512, (p + 1) * 512)
                tmp = smp.tile([128, 512], BF16, tag="rtmp", name="rtmp", bufs=4)
                nc.scalar.copy(tmp[0:64, :], ps[64:128, :])
                nc.vector.tensor_copy(tmp[64:128, :], ps[0:64, :])
                nc.vector.tensor_tensor(dst[:, sl], ps[:], cos_b[a][:, sl], op=ALU.mult)
                nc.gpsimd.tensor_tensor(tmp[:], tmp[:], sin_b[a][:, sl], op=ALU.mult)
                nc.vector.tensor_tensor(dst[:, sl], dst[:, sl], tmp[:], op=ALU.add)

            def v_block(wb):
                """v^T via wide matmuls, then one wide DMA transpose per half
                gives v in natural [pos, d] layout (v4[jg][:, jl*128+d])."""
                for jg in range(2):
                    ps = m1ps.tile([128, 512], F32, tag="m1", name="m1v")
                    for e in range(16):
                        nc.tensor.matmul(ps[:], wb[:, e * 128:(e + 1) * 128],
                                         xt[e][jg][:], start=(e == 0), stop=(e == 15))
                    vTh = smp.tile([128, 512], BF16, tag="vTh", name=f"vTh{jg}", bufs=2)
                    nc.vector.tensor_copy(vTh[:], ps[:])
                    nc.sync.dma_start(out=v4[jg][:, :, :], in_=vTh[:],
                                      transpose=True)

            # ---------------- attention pieces ----------------
            # per-stage small tiles (s0,s1,r0,r1,r1p as [128,8] col-per-i)
            def stage_tiles(s):
                return {k: smp.tile([128, NQT], F32, tag=k, name=f"{k}{s}")
                        for k in ("s0t", "s1t", "r0t", "r1t", "r1p")}

            def energy(h, i, e01):
                """PE: both halves' energy for q-block i, causal-masked via
                negU accumulate. e01 = (e0 tile, e1 tile) wide [128,1024]."""
                Ke = (i + 1) * 128
                nch = 1 if Ke <= 512 else 2
                dc = (i * 128) // 512  # chunk holding the diag block
                doff = i * 128
                for a in range(2):
                    et = e01[a]
                    for kc in range(nch):
                        w = min(Ke, (kc + 1) * 512) - kc * 512
                        ksl = slice(kc * 512, kc * 512 + w)
                        nc.tensor.matmul(
                            et[:, ksl],
                            qT[h][a][:, i * 128:(i + 1) * 128],
                            kT[a][:, ksl], start=True, stop=(kc != dc),
                            skip_group_check=True)
                    nc.tensor.matmul(et[:, doff:doff + 128], identb[:], negU[:],
                                     start=False, stop=True, skip_group_check=True)

            def exps01(i, e01, st):
                Ke = (i + 1) * 128
                # bufs=5: exp0(i) of the SECOND half-head must not reuse a
                # buffer still read by t(j<4), which waits on r1p(hh0) <-
                # accum of exp0(0..3) (bufs=3 deadlocks there)
                # bufs=7: within a stage only exp0(7) reuses a live buffer
                # (exp0(0)'s, whose reader t(0) completes early) so scalar
                # never stalls on the vector t-chain mid-stage
                exp0 = epool.tile([128, S], BF16, tag="exp0", name="exp0", bufs=7)
                exp1 = epool.tile([128, S], BF16, tag="exp1", name="exp1", bufs=7)
                nc.scalar.activation(exp0[:, 0:Ke], e01[0][:, 0:Ke], ACT.Exp,
                                     scale=ISCALE, accum_out=st["s0t"][:, i:i + 1])
                nc.scalar.activation(exp1[:, 0:Ke], e01[1][:, 0:Ke], ACT.Exp,
                                     scale=ISCALE, accum_out=st["s1t"][:, i:i + 1])
                return exp0, exp1

            def rchain(h, lo, n, st):
                """batched softmax scalars for i in [lo, lo+n):
                r0 = 1/s0, r1 = 1/s1, r1p = -lam*s0/s1"""
                cs = slice(lo, lo + n)
                nc.vector.reciprocal(st["r0t"][:, cs], st["s0t"][:, cs])
                nc.vector.reciprocal(st["r1t"][:, cs], st["s1t"][:, cs])
                nc.vector.scalar_tensor_tensor(
                    st["r1p"][:, cs], st["s0t"][:, cs], lam_t[:, h:h + 1],
                    st["r1t"][:, cs], op0=ALU.mult, op1=ALU.mult)

            def tmix(i, exp0, exp1, st):
                """t = exp0 + r1p*exp1, diag re-masked. Large-i mixes go to the
                otherwise-idle gpsimd engine (SBUF-only operands)."""
                Ke = (i + 1) * 128
                eng = nc.vector
                tt = epool.tile([128, S], BF16, tag="t", name="t", bufs=6)
                eng.scalar_tensor_tensor(
                    tt[:, 0:Ke], exp1[:, 0:Ke], st["r1p"][:, i:i + 1], exp0[:, 0:Ke],
                    op0=ALU.mult, op1=ALU.add)
                nc.vector.tensor_tensor(tt[:, i * 128:Ke], tt[:, i * 128:Ke],
                                        negU[:], op=ALU.add)
                return tt

            def exp2t(i, tt, st):
                Ke = (i + 1) * 128
                e2 = epool.tile([128, S], BF16, tag="exp2", name="exp2", bufs=4)
                nc.scalar.activation(e2[:, 0:Ke], tt[:, 0:Ke], ACT.Exp,
                                     scale=st["r0t"][:, i:i + 1])
                return e2

            def transpose_i(diffT, i, e2):
                Ke = (i + 1) * 128
                nc.sync.dma_start(out=diffT[:, i, 0:i + 1, :], in_=e2[:, 0:Ke],
                                  transpose=True)

            def pv_chunk(diffT, c):
                """PE: att^T psum for 512-q chunk c."""
                nk = 4 * c + 4
                attps = pvps.tile([128, 512], F32, tag="pv", name="att")
                for j in range(nk):
                    imin = max(4 * c, j)
                    off = (imin - 4 * c) * 128
                    jg, jl = j // 4, j % 4
                    nc.tensor.matmul(
                        attps[:, off:512], v4[jg][:, jl, :],
                        diffT[:, imin:4 * c + 4, j, :],
                        start=(j == 0), stop=(j == nk - 1))
                return attps

            def attr_copy(h, c, attps):
                nc.vector.tensor_copy(attr[h][c][:], attps[:])

            def att2_sq(h, c):
                a2 = smp.tile([128, 512], BF16, tag="att2", name="att2", bufs=2)
                nc.vector.tensor_tensor(a2[:], attr[h][c][:], attr[h][c][:],
                                        op=ALU.mult)
                return a2

            def ss_mm(h, c, a2):
                # per-q sum over head_dim: N=1 matmuls into the persistent
                # ssq strip; c-major layout so the c=0 RMS chain can run
                # before any c=1 PV exists (col = c*16 + h*4 + qb)
                for qb in range(4):
                    col = c * 16 + h * 4 + qb
                    nc.tensor.matmul(ssq[:, col:col + 1],
                                     a2[:, qb * 128:(qb + 1) * 128],
                                     ones_t[:, 0:1], start=True, stop=True)

            # ================= emission =================
            # --- pre-stage: k0,k1,v,q0 projections ---
            wb_k0 = load_wb(0, fine=True)
            load_x()
            wb_k1 = load_wb(1)
            wb_v = load_wb(2)
            load_consts()
            build_consts()
            for p in range(2):
                ps = proj_chunk(wb_k0, p)
                rope_chunk(kT[0], 0, ps, p)
            for p in range(2):
                ps = proj_chunk(wb_k1, p)
                rope_chunk(kT[1], 1, ps, p)
            v_block(wb_v)
            wb_q = [load_wb(3), load_wb(4)]
            for a in range(2):
                for p in range(2):
                    ps = proj_chunk(wb_q[a], p)
                    rope_chunk(qT[0][a], a, ps, p)

            # --- stages ---
            prev_pv = {}  # held tiles for the cross-stage PV chain
            diffTs = {}
            wops = opool = None
            for s in range(HPC):
                if s == HPC - 1:
                    # projections done: recycle x/w sbuf + the 2 proj psum
                    # banks for the output projection, which overlaps stage 3
                    pctx.close()
                    wops = ctx.enter_context(
                        tc.tile_pool(name="wops", bufs=2, space="PSUM"))
                    opool = ctx.enter_context(tc.tile_pool(name="osb", bufs=4))
                st = stage_tiles(s)
                diffT = dtpool.tile([128, NQT, NQT, 128], BF16, tag="difft",
                                    name=f"difft{s}")
                diffTs[s] = diffT
                qnext = s + 1 if s + 1 < HPC else None
                if qnext is not None:
                    wb_q = [load_wb(3 + 2 * qnext), load_wb(4 + 2 * qnext)]
                nc.sync.dma_start(out=wo_t[s][:], in_=Wo[s * 128:(s + 1) * 128, :])
                pvh = s - 1 if s >= 1 else None
                # in the last stage, defer PV(2) so the scheduler uses it to
                # fill the PE while stage-3's softmax chain completes
                defer_pv = (s == HPC - 1)

                e01s = {}

                def emit_e(i):
                    # single tag: e0/e1 share the pool's 2 wide buffers.
                    # For i<4 both halves fit in ONE wide tile (Ke<=512),
                    # which halves the ring pressure early in the stage.
                    if i < 4:
                        ew = engps.tile([128, S], F32, tag="e", name="e01")
                        e01s[i] = (ew[:, 0:512], ew[:, 512:1024])
                    else:
                        e0 = engps.tile([128, S], F32, tag="e", name="e0")
                        e1 = engps.tile([128, S], F32, tag="e", name="e1")
                        e01s[i] = (e0[:], e1[:])
                    energy(s, i, e01s[i])

                exps = {}
                tts = {}

                # interleaved PE emission with softmax chain on other engines
                emit_e(0)
                emit_e(1)
                if pvh is not None and not defer_pv:
                    attps0 = pv_chunk(diffTs[pvh], 0)
                    attr_copy(pvh, 0, attps0)
                exps[0] = exps01(0, e01s[0], st)
                exps[1] = exps01(1, e01s[1], st)
                emit_e(2)
                exps[2] = exps01(2, e01s[2], st)
                qps = {}
                if qnext is not None:
                    qps[(0, 0)] = proj_chunk(wb_q[0], 0)
                emit_e(3)
                exps[3] = exps01(3, e01s[3], st)
                rchain(s, 0, 4, st)
                for i in range(2):
                    tts[i] = tmix(i, *exps[i], st)
                if pvh is not None and not defer_pv:
                    a2 = att2_sq(pvh, 0)
                if qnext is not None:
                    qps[(0, 1)] = proj_chunk(wb_q[0], 1)
                for i in range(2, 4):
                    tts[i] = tmix(i, *exps[i], st)
                e2s = {}
                for i in range(2):
                    e2s[i] = exp2t(i, tts[i], st)
                    transpose_i(diffT, i, e2s[i])
                if pvh is not None and not defer_pv:
                    ss_mm(pvh, 0, a2)
                if s == HPC - 1:
                    # last stage: finish the i<4 exp2/transposes BEFORE the
                    # i>=4 exps so PV(3,0) (which gates out-proj q<512)
                    # unblocks ~3us earlier; chainB hides under those outs
                    for i in range(2, 4):
                        e2s[i] = exp2t(i, tts[i], st)
                        transpose_i(diffT, i, e2s[i])
                emit_e(4)
                exps[4] = exps01(4, e01s[4], st)
                if s != HPC - 1:
                    for i in range(2, 4):
                        e2s[i] = exp2t(i, tts[i], st)
                        transpose_i(diffT, i, e2s[i])
                if qnext is not None:
                    with tc.high_priority(offset=200):
                        rope_chunk(qT[qnext][0], 0, qps[(0, 0)], 0)
                emit_e(5)
                exps[5] = exps01(5, e01s[5], st)
                if qnext is not None:
                    qps[(1, 0)] = proj_chunk(wb_q[1], 0)
                if pvh is not None and not defer_pv:
                    attps1 = pv_chunk(diffTs[pvh], 1)
                    attr_copy(pvh, 1, attps1)
                emit_e(6)
                exps[6] = exps01(6, e01s[6], st)
                if qnext is not None:
                    with tc.high_priority(offset=200):
                        rope_chunk(qT[qnext][0], 0, qps[(0, 1)], 1)
                # quarter-head batching: i=4,5 scalars right after exp1(5)
                # so their t/exp2/transpose overlap exps(6,7)
                rchain(s, 4, 2, st)
                tts[4] = tmix(4, *exps[4], st)
                e2s[4] = exp2t(4, tts[4], st)
                transpose_i(diffT, 4, e2s[4])
                emit_e(7)
                exps[7] = exps01(7, e01s[7], st)
                tts[5] = tmix(5, *exps[5], st)
                e2s[5] = exp2t(5, tts[5], st)
                transpose_i(diffT, 5, e2s[5])
                rchain(s, 6, 2, st)
                if qnext is not None:
                    qps[(1, 1)] = proj_chunk(wb_q[1], 1)
                if pvh is not None and not defer_pv:
                    a2b = att2_sq(pvh, 1)
                    ss_mm(pvh, 1, a2b)
                tts[6] = tmix(6, *exps[6], st)
                e2s[6] = exp2t(6, tts[6], st)
                transpose_i(diffT, 6, e2s[6])
                if qnext is not None:
                    with tc.high_priority(offset=200):
                        rope_chunk(qT[qnext][1], 1, qps[(1, 0)], 0)
                tts[7] = tmix(7, *exps[7], st)
                e2s[7] = exp2t(7, tts[7], st)
                transpose_i(diffT, 7, e2s[7])
                if qnext is not None:
                    with tc.high_priority(offset=200):
                        rope_chunk(qT[qnext][1], 1, qps[(1, 1)], 1)

                if pvh is not None and defer_pv:
                    attps0 = pv_chunk(diffTs[pvh], 0)
                    attr_copy(pvh, 0, attps0)
                    ss_mm(pvh, 0, att2_sq(pvh, 0))
                    attps1 = pv_chunk(diffTs[pvh], 1)
                    attr_copy(pvh, 1, attps1)
                    ss_mm(pvh, 1, att2_sq(pvh, 1))

            # --- tail: PV(3) + split RMS + output projection ---
            # chain for one c-group: compact recip+sqrt on 16 ssq cols,
            # pad-transpose to rows, selector-matmul broadcast, attf multiply
            attf = {}

            def rms_head(c):
                cs = slice(c * 16, c * 16 + 16)
                rq = opool.tile([128, 16], F32, tag="rq", name=f"rq{c}", bufs=2)
                nc.vector.reciprocal(rq[:], ssq[:, cs])
                cfq = opool.tile([128, 128], BF16, tag="cfq", name=f"cfq{c}", bufs=2)
                nc.gpsimd.memset(cfq[:], 0.0)  # unused cols are still transposed
                nc.scalar.activation(cfq[:, cs], rq[:], ACT.Sqrt, scale=128.0)
                cfrows = opool.tile([128, 128], BF16, tag="cfrows",
                                    name=f"cfrows{c}", bufs=2)
                nc.sync.dma_start(out=cfrows[:, :], in_=cfq[:], transpose=True)
                return cfrows

            def rms_bcast(c, cfrows):
                for h in range(HPC):
                    cb = pvps.tile([128, 512], F32, tag="pv", name=f"cb{h}{c}")
                    for qb in range(4):
                        col = c * 16 + h * 4 + qb
                        nc.tensor.matmul(cb[:, qb * 128:(qb + 1) * 128],
                                         sel[:, col * 128:(col + 1) * 128],
                                         cfrows[0:32, :], start=True, stop=True)
                    af = opool.tile([128, 512], BF16, tag="attf", name=f"af{h}{c}", bufs=8)
                    nc.vector.tensor_tensor(af[:], attr[h][c][:], cb[:], op=ALU.mult)
                    attf[(h, c)] = af

            def emit_out(p):
                # 4 accumulators per position block: the energy pool's wide
                # tiles are dead after stage 3's exps, so each gives 2 banks;
                # engps bufs=2 also pipelines p against p+1's drain
                c, po = p // 4, (p % 4) * 128
                ew = engps.tile([128, S], F32, tag="e", name=f"ow{p}")
                ops = [ew[:, 0:512], ew[:, 512:1024],
                       wops.tile([128, 512], F32, tag="o", name="o")[:],
                       wops.tile([128, 512], F32, tag="o", name="o")[:]]
                for h in range(HPC):
                    for n in range(E // 512):
                        nc.tensor.matmul(
                            ops[n], attf[(h, c)][:, po:po + 128],
                            wo_t[h][:, n * 512:(n + 1) * 512],
                            start=(h == 0), stop=(h == HPC - 1))
                # wide osb: psum banks free after the engine copies either
                # way; one [128,2048] store replaces 4 issues (+fewer sems)
                osb = opool.tile([128, E], BF16, tag="osb", name="osb", bufs=2)
                for n in range(E // 512):
                    if n % 2 == 0:
                        nc.vector.tensor_copy(osb[:, n * 512:(n + 1) * 512], ops[n])
                    else:
                        nc.scalar.copy(osb[:, n * 512:(n + 1) * 512], ops[n])
                dq = nc.gpsimd if p % 2 == 0 else nc.sync
                dq.dma_start(out=out_ext[p * 128:(p + 1) * 128, :], in_=osb[:])

            # c=0 group: PV(3,0) only needs diffT(3) blocks i<4 (mid-stage-3),
            # so out-proj for q<512 overlaps the rest of stage 3
            attps = pv_chunk(diffTs[HPC - 1], 0)
            attr_copy(HPC - 1, 0, attps)
            ss_mm(HPC - 1, 0, att2_sq(HPC - 1, 0))
            cfrows0 = rms_head(0)
            rms_bcast(0, cfrows0)
            # the whole c=1 chain is emitted BEFORE the out loops so its
            # cross-engine latency hides under the p<4 projections
            attps = pv_chunk(diffTs[HPC - 1], 1)
            attr_copy(HPC - 1, 1, attps)
            ss_mm(HPC - 1, 1, att2_sq(HPC - 1, 1))
            cfrows1 = rms_head(1)
            rms_bcast(1, cfrows1)
            for p in range(NQT):
                emit_out(p)

    nc.finalize()
    return nc


def _host_prep(x, Wq, Wk, Wv, Wo, lq1, lq2, lk1, lk2, rms_w):
    lam = (np.exp((lq1 * lk1).sum(-1)) - np.exp((lq2 * lk2).sum(-1))
           + LAM_INIT).astype(np.float32)  # (H,)
    j = np.arange(D, dtype=np.float64)
    theta = 1.0 / (10000.0 ** (2.0 * j / (2 * D)))
    pos = np.arange(S, dtype=np.float64)
    ang = pos[None, :] * theta[:, None]  # (128, S)
    cosd = np.cos(ang).astype(np.float32)
    sin = np.sin(ang)
    cosd2 = np.concatenate([np.concatenate([cosd[a * 64:(a + 1) * 64]] * 2, 0)
                            for a in range(2)], 0)
    sind2 = np.concatenate(
        [np.concatenate([-sin[a * 64:(a + 1) * 64], sin[a * 64:(a + 1) * 64]], 0)
         for a in range(2)], 0).astype(np.float32)

    perm256 = np.concatenate([np.arange(0, 128, 2), np.arange(1, 128, 2),
                              np.arange(128, 256, 2), np.arange(129, 256, 2)])
    Wqp = Wq.reshape(E, H, 2 * D)[:, :, perm256].reshape(E, H * 2 * D)
    Wkp = Wk.reshape(E, KVH, 2 * D)[:, :, perm256].reshape(E, KVH * 2 * D)
    WoS = (Wo.reshape(H, D, E) * (rms_w[None, :, None] * (1.0 - LAM_INIT))
           ).reshape(E, E).astype(np.float32)

    maskn = np.where(np.arange(128)[None, :] > np.arange(128)[:, None],
                     np.float32(NEG), np.float32(0.0)).astype(np.float32)

    def interleave(Wcols):
        # [E, nb*128] -> [128, nb*2048]; block b, chunk e at cols b*2048+e*128
        nb = Wcols.shape[1] // 128
        return np.ascontiguousarray(
            Wcols.reshape(16, 128, nb, 128).transpose(1, 2, 0, 3)
            .reshape(128, nb * 2048))

    import ml_dtypes
    bf = ml_dtypes.bfloat16
    in_maps = []
    for core in range(NCORES):
        b, g = divmod(core, KVH)
        heads = slice(HPC * g * 2 * D, HPC * (g + 1) * 2 * D)
        lam_g = lam[HPC * g:HPC * (g + 1)]
        # block order must match kernel: k0,k1,v,q0..q7
        Wcat = np.concatenate([
            Wkp[:, g * 2 * D:(g + 1) * 2 * D],
            Wv[:, g * D:(g + 1) * D],
            Wqp[:, heads]], axis=1)
        in_maps.append({
            "xT": np.ascontiguousarray(
                x[b].T.reshape(E, 2, 512).transpose(1, 0, 2).reshape(2 * E, 512)
            ).astype(bf),
            "Wil": interleave(Wcat).astype(bf),
            "Wo": np.ascontiguousarray(WoS[HPC * D * g:HPC * D * (g + 1), :]).astype(bf),
            "cosd": cosd2.astype(bf),
            "sind": sind2.astype(bf),
            "lamn": np.tile(-lam_g[None, :], (D, 1)).astype(np.float32),
            "maskn": maskn,
        })
    return in_maps


def kernel(x, Wq, Wk, Wv, Wo, lq1, lq2, lk1, lk2, rms_w, _trace=False):
    from concourse import bass_utils

    in_maps = _host_prep(np.asarray(x, np.float32), np.asarray(Wq, np.float32),
                         np.asarray(Wk, np.float32), np.asarray(Wv, np.float32),
                         np.asarray(Wo, np.float32), np.asarray(lq1, np.float32),
                         np.asarray(lq2, np.float32), np.asarray(lk1, np.float32),
                         np.asarray(lk2, np.float32), np.asarray(rms_w, np.float32))
    if "nc" not in _cache:
        _cache["nc"] = _build()
    nc = _cache["nc"]
    res = bass_utils.run_bass_kernel_spmd(
        nc, in_maps, core_ids=list(range(NCORES)), trace=_trace)
    _cache["last_result"] = res
    parts = np.stack([np.asarray(res.results[c]["out"], dtype=np.float32)
                      for c in range(NCORES)], 0)
    out = parts.reshape(B, KVH, S, E).sum(1)
    return out.astype(np.float32)
